# revision 2
# baseline (speedup 1.0000x reference)
"""CMoE hash-routed expert FFN on 8 NeuronCores (expert-parallel).

Host side (the shard/unshard steps): compute hash routing
e = (token_id % 5099) % 64, first-come slot assignment with capacity 512,
scatter tokens into a per-expert [E, D, C] buffer (transposed, bf16), and
shard 8 experts to each of the 8 cores along with that core's (transposed,
bf16) expert weights.  Device side: per expert
    h  = relu(A @ Wk^T)^2        [C, F]
    kv = h @ Wv^T                [C, D]
    r  = sigmoid(A @ Wr^T)       [C, D]
    out = r * kv
computed entirely in transposed form (contraction dim on SBUF partitions),
bf16 matmul operands with fp32 PSUM accumulation.  Host gathers each
token's slot back out of [E, D, C] and zeroes dropped tokens.

DMA discipline: transfers are issued in strict first-need order, chunked so
compute gates on partial tensors (wk in 4 f-chunks, wv in 2 fo-chunks), and
spread over three rings (sync HWDGE: at+wv, scalar HWDGE: wk, gpsimd SWDGE:
wr + output stores).  The r phase of expert e+1 is emitted after kv(e); its
at/wr loads are issued two experts ahead.  Outputs are stored as bf16 to
halve store traffic; the host upcasts.
"""

import numpy as np
import ml_dtypes

import concourse.bass as bass
import concourse.mybir as mybir
import concourse.tile as tile
from concourse import bacc
from concourse.bass import ts
from concourse.bass_utils import run_bass_kernel_spmd

HASH_PRIME = 5099
B, T, D, F, E = 8, 4096, 512, 1792, 64
S = B * T
C = 512  # capacity = max(4, ceil(S/E))
N_CORES = 8
E_LOC = E // N_CORES  # experts per core

BF16 = mybir.dt.bfloat16
F32 = mybir.dt.float32

_NC = None  # cached compiled Bass program
LAST_RESULT = None  # BassKernelResults of the most recent run (for test.py)


def _build_nc(e_loc=E_LOC, d=D, f=F, c=C):
    """One SPMD program: each core computes e_loc experts' FFN."""
    kd = d // 128   # contraction tiles over D
    kf = f // 128   # contraction tiles over F
    nc = bacc.Bacc("TRN2", target_bir_lowering=False, debug=False,
                   num_devices=N_CORES)

    a_t = nc.dram_tensor("a_t", [e_loc, d, c], BF16, kind="ExternalInput")
    wk_t = nc.dram_tensor("wk_t", [e_loc, d, f], BF16, kind="ExternalInput")
    wr_t = nc.dram_tensor("wr_t", [e_loc, d, d], BF16, kind="ExternalInput")
    wv_t = nc.dram_tensor("wv_t", [e_loc, f, d], BF16, kind="ExternalInput")
    out_t = nc.dram_tensor("out_t", [e_loc, d, c], BF16, kind="ExternalOutput")

    with tile.TileContext(nc) as tc:
        with (
            tc.tile_pool(name="wts", bufs=2) as wts,
            tc.tile_pool(name="acts", bufs=2) as acts,
            tc.tile_pool(name="ph", bufs=3, space="PSUM") as ph,
            tc.tile_pool(name="pr", bufs=3, space="PSUM") as pr,
            tc.tile_pool(name="pkv", bufs=2, space="PSUM") as pkv,
        ):
            tiles_awr = {}
            tiles_wk = {}
            tiles_wv = {}
            sigs = {}
            hbs = {}

            # Warm the PE (HAM throttles it to 1.2 GHz until ~3.4us of
            # sustained work) with matmuls on scratch data while the first
            # input DMAs ramp up; the result is never read.  8 cold MMs
            # bridge ~3.4us, landing right when at0/wr0 arrive.
            warm_l = wts.tile([128, 128], BF16, tag="warm_l")
            warm_r = wts.tile([128, c], BF16, tag="warm_r")
            nc.any.memset(warm_l[:], 0.0)
            nc.any.memset(warm_r[:], 0.0)
            for _ in range(8):
                warm_p = pr.tile([128, c], F32, tag="psr")
                nc.tensor.matmul(warm_p[:], lhsT=warm_l[:], rhs=warm_r[:],
                                 start=True, stop=True)

            def load_at_wr(e):
                at = wts.tile([128, kd, c], BF16, tag="at")
                wr = wts.tile([128, kd, d], BF16, tag="wr")
                tiles_awr[e] = (at, wr)
                nc.sync.dma_start(at[:], a_t[e].rearrange("(ko p) c -> p ko c", p=128))
                wr_src = wr_t[e].rearrange("(ko p) g -> p ko g", p=128)
                # gpsimd ring is otherwise idle of loads; keeps wr off the
                # sync ring's critical path
                nc.gpsimd.dma_start(wr[:], wr_src)

            def load_wk(e):
                wk = wts.tile([128, kd, f], BF16, tag="wk")
                tiles_wk[e] = wk
                src = wk_t[e].rearrange("(ko p) f -> p ko f", p=128)
                # 4 chunks: h ft-groups gate on the 448-col chunk they read
                for lo in range(0, f, 512):
                    hi = min(lo + 512, f)
                    nc.scalar.dma_start(wk[:, :, lo:hi], src[:, :, lo:hi])

            def load_wv(e):
                wv = wts.tile([128, kf, d], BF16, tag="wv")
                tiles_wv[e] = wv
                src = wv_t[e].rearrange("(fo p) g -> p fo g", p=128)
                half = kf // 2
                nc.sync.dma_start(wv[:, :half, :], src[:, :half, :])
                nc.sync.dma_start(wv[:, half:, :], src[:, half:, :])

            def emit_r(e):
                at, wr = tiles_awr[e]
                sig = acts.tile([128, kd, c], F32, tag="sig")
                sigs[e] = sig
                for gt in range(kd):
                    psum_r = pr.tile([128, c], F32, tag="psr")
                    for kt in range(kd):
                        nc.tensor.matmul(
                            psum_r[:],
                            lhsT=wr[:, kt, ts(gt, 128)],
                            rhs=at[:, kt, :],
                            start=(kt == 0),
                            stop=(kt == kd - 1),
                        )
                    nc.scalar.activation(sig[:, gt, :], psum_r[:],
                                         mybir.ActivationFunctionType.Sigmoid)

            def emit_h(e):
                at, _ = tiles_awr[e]
                wk = tiles_wk.pop(e)
                # h^T[f, c] = (relu(Wk^T.T @ A^T))^2, bf16 for matmul 2
                hb = acts.tile([128, kf, c], BF16, tag="hb")
                hbs[e] = hb
                for ft in range(kf):
                    psum_h = ph.tile([128, c], F32, tag="psh")
                    for kt in range(kd):
                        nc.tensor.matmul(
                            psum_h[:],
                            lhsT=wk[:, kt, ts(ft, 128)],
                            rhs=at[:, kt, :],
                            start=(kt == 0),
                            stop=(kt == kd - 1),
                        )
                    nc.scalar.activation(hb[:, ft, :], psum_h[:],
                                         mybir.ActivationFunctionType.Relu)
                    nc.vector.tensor_mul(hb[:, ft, :], hb[:, ft, :], hb[:, ft, :])

            def emit_kv(e):
                tiles_awr.pop(e)
                wv = tiles_wv.pop(e)
                hb = hbs.pop(e)
                sig = sigs.pop(e)
                # kv^T[dd, c] = Wv^T.T @ h^T ; out = sig * kv
                ob = acts.tile([128, kd, c], BF16, tag="ob")
                for dt in range(kd):
                    psum_kv = pkv.tile([128, c], F32, tag="pskv")
                    for ft in range(kf):
                        nc.tensor.matmul(
                            psum_kv[:],
                            lhsT=wv[:, ft, ts(dt, 128)],
                            rhs=hb[:, ft, :],
                            start=(ft == 0),
                            stop=(ft == kf - 1),
                        )
                    nc.vector.tensor_mul(ob[:, dt, :], psum_kv[:], sig[:, dt, :])
                    # store each d-tile as it finishes; the last expert's
                    # stores ride the by-then-idle sync HWDGE ring (lower
                    # latency than SWDGE) to shorten the kernel tail
                    dst = out_t[e].rearrange("(ko p) c -> p ko c", p=128)[:, dt, :]
                    if e == e_loc - 1:
                        nc.sync.dma_start(dst, ob[:, dt, :])
                    else:
                        nc.gpsimd.dma_start(dst, ob[:, dt, :])

            # DMAs are issued in strict first-need order; compute for
            # expert e is emitted as r(e) | h(e) kv(e), with r one expert
            # ahead of h/kv so the PE always has work while wk/wv stream.
            load_at_wr(0)
            load_wk(0)
            emit_r(0)
            load_wv(0)
            if e_loc > 1:
                load_at_wr(1)
                load_wk(1)
            for e in range(e_loc):
                emit_h(e)
                emit_kv(e)
                if e + 1 < e_loc:
                    emit_r(e + 1)
                    load_wv(e + 1)
                if e + 2 < e_loc:
                    load_at_wr(e + 2)
                    load_wk(e + 2)

    nc.compile()
    return nc


def _route(token_ids):
    tid = token_ids.reshape(S).astype(np.int64)
    e_idx = (tid % HASH_PRIME) % E
    order = np.argsort(e_idx, kind="stable")
    sorted_e = e_idx[order]
    starts = np.searchsorted(sorted_e, np.arange(E))
    pos = np.empty(S, np.int64)
    pos[order] = np.arange(S) - starts[sorted_e]
    kept = pos < C
    return e_idx, pos, kept


def kernel(x, token_ids, Wk, Wr, Wv):
    global _NC, LAST_RESULT
    if _NC is None:
        _NC = _build_nc()

    e_idx, pos, kept = _route(token_ids)

    bf16 = ml_dtypes.bfloat16
    xf = np.ascontiguousarray(x, dtype=np.float32).reshape(S, D)
    disp_t = np.zeros((E, D, C), np.float32)
    disp_t[e_idx[kept], :, pos[kept]] = xf[kept]
    a_t = disp_t.astype(bf16)

    wk_t = np.asarray(Wk, dtype=np.float32).transpose(0, 2, 1).astype(bf16)
    wr_t = np.asarray(Wr, dtype=np.float32).transpose(0, 2, 1).astype(bf16)
    wv_t = np.asarray(Wv, dtype=np.float32).transpose(0, 2, 1).astype(bf16)

    in_maps = [
        {
            "a_t": a_t[i * E_LOC:(i + 1) * E_LOC],
            "wk_t": wk_t[i * E_LOC:(i + 1) * E_LOC],
            "wr_t": wr_t[i * E_LOC:(i + 1) * E_LOC],
            "wv_t": wv_t[i * E_LOC:(i + 1) * E_LOC],
        }
        for i in range(N_CORES)
    ]

    LAST_RESULT = run_bass_kernel_spmd(_NC, in_maps, list(range(N_CORES)))
    out_t = np.concatenate(
        [np.asarray(LAST_RESULT.results[i]["out_t"]).astype(np.float32)
         for i in range(N_CORES)], axis=0)

    yf = out_t[e_idx, :, np.minimum(pos, C - 1)]
    yf[~kept] = 0.0
    return np.ascontiguousarray(yf.reshape(B, T, D), dtype=np.float32)
